# revision 39
# baseline (speedup 1.0000x reference)
"""MixedMoE Trainium2 kernel: sparse expert routing over 8 NeuronCores.

Reference computation (top-2 of 16 experts, combine weight c[t,e] = softmax
score if e in top-2 else exactly 0):
    emb = embeddings.reshape(T, D)
    experts 0..1 consume x, experts 2..15 consume emb (SwiGLU, inter dim H)
    y[t] = sum_e c[t,e] * expert_e(...)[t]          (c exactly 0 off top-2)
    z = silu(emb @ sW1 + sB1) @ sW2 + sB2           (shared experts, all tokens)
    out = (y + z).reshape(B, S, D)

Because c is exactly zero off the top-2, skipping non-routed (token, expert)
pairs matches the dense reference: we only drop terms that are 0.0 * finite.
The host computes the gate (0.03% of the FLOPs), gathers each expert's routed
tokens, and scatters the expert outputs back.

Sharding (SPMD, one program, per-core data):
  core c holds routed experts {2c, 2c+1}; the host gathers each expert's
  routed tokens (padded to a common capacity C at 64-slot granularity; pad
  slots have c=0) into a [D, C] activation block. The shared experts are
  token-sharded: core c computes the full 2048-wide shared MLP for tokens
  [512c, 512c+512) of emb.

All matmul operands are bf16 (end-to-end error ~3e-3 << the 2e-2 gate):
same 1 cycle/row as f32r, but FWL halves LDWEIGHTS and all DMA halves.
Outputs are bf16 too (host upcasts; the exact-linear bias terms and the
final scatter-add stay fp32 on host). A burst of dummy matmuls at kernel
start warms the PE HAM clock gate while the first DMAs stream; the first
moving block is only 128 columns so real matmuls start early. Outputs go
out over the scalar/gpsimd queues while sync streams weights.
"""

import numpy as np
import ml_dtypes

B_DIM, S_DIM, D = 4, 1024, 1024
T = B_DIM * S_DIM  # 4096 tokens
H = 1024  # routed expert inter dim
E = 16
N_CORES = 8
E_LOC = 2  # routed experts per core
SH = 2048  # shared experts inter dim
SH_T = SH // 128  # 16 shared h-tiles
TS = T // N_CORES  # 512 shared tokens per core
HT = H // 128  # 8 h-tiles per routed expert
D_T = D // 128  # 8 k-tiles in D

BF16 = ml_dtypes.bfloat16

_CACHED = {}  # C -> compiled nc
LAST_IN_MAPS = None  # kept for external timing/debug harnesses


def _tsubs_for(C):
    """Stage-2 token subtiles: 128-wide, plus a trailing 64-wide if C%128."""
    out = [128] * (C // 128)
    if C % 128:
        out.append(64)
    return out


def _widths_for(chunk, head):
    """Stage-1 moving-dim pieces, each <=512 (PSUM bank limit for fp32
    accumulation). All pieces except the last are multiples of 128 so
    stage-2 token subtiles never straddle a piece. The first expert's
    pieces start small ([128, 128, ...]) so the first matmuls only wait
    on ~200KB of DMA and later groups unblock incrementally while the
    activation stream is still ramping."""
    if head and chunk > 128:
        out = [128]
        chunk -= 128
    else:
        out = []
    while chunk > 512:
        out.append(512)
        chunk -= 512
    if chunk:
        out.append(chunk)
    return out


def _shrink_redundant_ldw(nc, mybir):
    """Shrink back-to-back PE weight loads of the same stationary tile.

    bass emits one InstLdweights per matmul. When consecutive matmuls share
    the stationary operand (only matmuls in between), the repeat load is
    redundant -- the PE array already holds those exact weights. The repeat
    instruction must stay (it carries the DMA-wait semaphores that
    move_matmul_waits_to_ldweights later attaches), so instead of deleting
    it we shrink its access pattern to a single column: it reloads PE
    column 0 with identical values (a no-op by value) in ~1 cycle instead
    of 64.
    """
    PE = mybir.EngineType.PE
    n = 0
    for blk in nc.m.functions[0].blocks:
        last_key = None
        for inst in blk.instructions:
            if isinstance(inst, mybir.InstLdweights) and inst.engine == PE:
                ap = inst.ins[0]
                key = str(ap)
                if key == last_key:
                    shr = mybir.PhysicalAccessPattern(
                        kind="physical_ap",
                        ap=[list(ap.ap[0]), [1, 1]],
                        offset=ap.offset, dtype=ap.dtype,
                        memref=ap.memref, memsetref=ap.memsetref,
                    )
                    inst.ins = [shr]
                    n += 1
                else:
                    last_key = key
            elif isinstance(inst, mybir.InstMatmult) and inst.engine == PE:
                pass  # matmuls between identical loads keep the weights hot
            elif inst.engine == PE or isinstance(
                inst, (mybir.InstUnconditionalBranch, mybir.InstCall)
            ):
                last_key = None
    return n


def _build(C):
    import concourse.tile as tile
    from concourse import bacc, mybir

    f32 = mybir.dt.float32
    bf16 = mybir.dt.bfloat16
    SILU = mybir.ActivationFunctionType.Silu
    NT = len(_tsubs_for(C))  # t-subtiles per routed expert

    nc = bacc.Bacc(trn_type="TRN2")

    # ---- DRAM I/O ----
    # activations are stored tile-contiguous (one flat [128*w] block per
    # (si, dt) SBUF tile, in consumption order) so every activation DMA is
    # a single fat contiguous transfer instead of 128 sub-2KB strided lines
    bt0_d = nc.dram_tensor("bt0", [D * C], bf16, kind="ExternalInput")
    bt1_d = nc.dram_tensor("bt1", [D * C], bf16, kind="ExternalInput")
    at_d = nc.dram_tensor("at", [D * TS], bf16, kind="ExternalInput")
    # W1/W3 pre-laid-out per (expert, h_tile): [e, ht, p, dt, h] so each
    # [128, 8, 128] SBUF tile is one fully-contiguous DRAM block
    w1_d = nc.dram_tensor("w1", [E_LOC, HT, 128, 8, 128], bf16, kind="ExternalInput")
    w3_d = nc.dram_tensor("w3", [E_LOC, HT, 128, 8, 128], bf16, kind="ExternalInput")
    w2_d = nc.dram_tensor("w2", [E_LOC, H, D], bf16, kind="ExternalInput")
    sw1_d = nc.dram_tensor("sw1", [SH_T, 128, 8, 128], bf16, kind="ExternalInput")
    sw2_d = nc.dram_tensor("sw2", [SH, D], bf16, kind="ExternalInput")
    # combine scalars csc[p, e*NT + j] = c[token in slot j*128+p, expert e]
    csc_d = nc.dram_tensor("csc", [128, E_LOC * NT], f32, kind="ExternalInput")
    b1_d = nc.dram_tensor("b1", [128, E_LOC * HT], f32, kind="ExternalInput")
    sb1_d = nc.dram_tensor("sb1", [128, SH_T], f32, kind="ExternalInput")
    out_d = nc.dram_tensor("out", [E_LOC * C + TS, D], bf16, kind="ExternalOutput")

    with tile.TileContext(nc) as tc:
        with (
            tc.tile_pool(name="small", bufs=1) as small,
            tc.tile_pool(name="btp", bufs=5) as btp,
            tc.tile_pool(name="w13p", bufs=6) as w13p,
            tc.tile_pool(name="w2p", bufs=17) as w2p,
            tc.tile_pool(name="htp", bufs=18) as htp,
            tc.tile_pool(name="silup", bufs=3) as silup,
            tc.tile_pool(name="yp", bufs=5) as ypool,
            tc.tile_pool(name="ps1", bufs=4, space="PSUM") as ps1,
            tc.tile_pool(name="ps2", bufs=4, space="PSUM") as ps2,
        ):
            csc = small.tile([128, E_LOC * NT], f32)
            b1 = small.tile([128, E_LOC * HT], f32)
            sb1 = small.tile([128, SH_T], f32)
            # HAM warm-up: zeroed operands, dummy matmuls into a scratch
            # PSUM tile; runs while the first DMAs stream so the PE clock
            # gate is already at 2.4 GHz when the real matmuls arrive.
            wu_w = small.tile([128, 128], bf16)
            wu_m = small.tile([128, 512], bf16)
            nc.vector.memset(wu_w[:], 0)
            nc.vector.memset(wu_m[:], 0)
            # 16 x 512: ~3.41us at the cold 1.2 GHz clock un-throttles the
            # HAM clock gate (one full window), and the remaining warm
            # matmuls bridge the PE over the DMA-bound head so the real
            # stream starts warm and nearly gap-free.
            for _ in range(16):
                wu_p = ps2.tile([128, 512], f32, tag="acc")
                nc.tensor.matmul(wu_p[:], lhsT=wu_w[:], rhs=wu_m[:],
                                 start=True, stop=True)

            def load_acts(dram, widths):
                # one [128, 8, w] tile and ONE fat DMA per si block: a
                # single trigger instruction on the scalar ring (each
                # trigger costs ~0.6us of engine time) and one maximally
                # contiguous HBM read
                tiles = []
                off = 0
                for w in widths:
                    t = btp.tile([128, D_T, 512], bf16, tag="bt")
                    nc.scalar.dma_start(
                        t[:, :, :w], dram[off : off + 128 * D_T * w]
                    )
                    tiles.append(t)
                    off += 128 * D_T * w
                return tiles

            def smalls_once():
                nc.sync.dma_start(sb1[:], sb1_d[:])
                nc.sync.dma_start(csc[:], csc_d[:])
                nc.sync.dma_start(b1[:], b1_d[:])

            first = True
            # ---- routed expert phases ----
            for e in range(E_LOC):
                bt_d = (bt0_d, bt1_d)[e]
                widths = _widths_for(C, head=(e == 0))
                if first:
                    smalls_once()
                    # first h-tile's W1/W3 ahead of the activations so the
                    # first real matmul waits on ~300KB, not ~1.5MB
                    w1s_f = w13p.tile([128, 8, 128], bf16, tag="w13")
                    nc.sync.dma_start(w1s_f[:], w1_d[0, 0])
                    w3s_f = w13p.tile([128, 8, 128], bf16, tag="w13")
                    nc.sync.dma_start(w3s_f[:], w3_d[0, 0])
                bts = load_acts(bt_d, widths)
                hts = [[None] * len(widths) for _ in range(HT)]
                w2s = []
                for ht in range(HT):
                    # W2 for this h-tile ahead of its W1/W3 in the scalar
                    # FIFO: issues ~2 h-tile periods early, so stage 2
                    # never waits on the last W2 transfer
                    w2t = w2p.tile([128, D], bf16, tag="w2")
                    nc.scalar.dma_start(
                        w2t[:], w2_d[e, ht * 128 : (ht + 1) * 128, :]
                    )
                    w2s.append(w2t)
                    if first and ht == 0:
                        w1s, w3s = w1s_f, w3s_f
                        first = False
                    else:
                        w1s = w13p.tile([128, 8, 128], bf16, tag="w13")
                        nc.sync.dma_start(w1s[:], w1_d[e, ht])
                        w3s = w13p.tile([128, 8, 128], bf16, tag="w13")
                        nc.sync.dma_start(w3s[:], w3_d[e, ht])
                    for si, w in enumerate(widths):
                        u1 = ps1.tile([128, 512], f32, tag="u")
                        u3 = ps1.tile([128, 512], f32, tag="u")
                        for dt in range(D_T):
                            nc.tensor.matmul(
                                u1[:, :w], lhsT=w1s[:, dt, :],
                                rhs=bts[si][:, dt, :w],
                                start=(dt == 0), stop=(dt == D_T - 1),
                            )
                        for dt in range(D_T):
                            nc.tensor.matmul(
                                u3[:, :w], lhsT=w3s[:, dt, :],
                                rhs=bts[si][:, dt, :w],
                                start=(dt == 0), stop=(dt == D_T - 1),
                            )
                        sil = silup.tile([128, 512], f32, tag="sil")
                        nc.scalar.activation(
                            sil[:, :w], u1[:, :w], SILU,
                            bias=b1[:, e * HT + ht : e * HT + ht + 1],
                        )
                        hx = htp.tile([128, 512], bf16, tag="ht")
                        nc.vector.tensor_mul(hx[:, :w], sil[:, :w], u3[:, :w])
                        hts[ht][si] = hx

                # tsub -> (sub index, col offset inside that sub)
                tmap, pos = [], 0
                for tw in _tsubs_for(C):
                    acc_w, si = 0, 0
                    while acc_w + widths[si] <= pos:
                        acc_w += widths[si]
                        si += 1
                    tmap.append((si, pos - acc_w, tw))
                    pos += tw
                for tsub, (si, off, tw) in enumerate(tmap):
                    g = e * NT + tsub
                    yt = ypool.tile([128, D], bf16, tag="y")
                    for dch in range(D // 512):
                        acc = ps2.tile([128, 512], f32, tag="acc")
                        for ht in range(HT):
                            nc.tensor.matmul(
                                acc[:tw, :],
                                lhsT=hts[ht][si][:, off : off + tw],
                                rhs=w2s[ht][:, dch * 512 : (dch + 1) * 512],
                                start=(ht == 0), stop=(ht == HT - 1),
                            )
                        nc.vector.tensor_scalar_mul(
                            yt[:tw, dch * 512 : (dch + 1) * 512],
                            acc[:tw, :], csc[:tw, g : g + 1],
                        )
                    row = e * C + tsub * 128
                    nc.gpsimd.dma_start(out_d[row : row + tw, :], yt[:tw, :])

            # ---- shared expert phase (512 tokens, full 2048 hidden) ----
            ats = load_acts(at_d, [512])
            sw2s = []
            hsh = [None] * SH_T
            for ht in range(SH_T):
                w2t = w2p.tile([128, D], bf16, tag="w2")
                nc.sync.dma_start(w2t[:], sw2_d[ht * 128 : (ht + 1) * 128, :])
                sw2s.append(w2t)
                w1s = w13p.tile([128, 8, 128], bf16, tag="w13")
                nc.sync.dma_start(w1s[:], sw1_d[ht])
                u1 = ps1.tile([128, 512], f32, tag="u")
                for dt in range(D_T):
                    nc.tensor.matmul(
                        u1[:], lhsT=w1s[:, dt, :], rhs=ats[0][:, dt, :],
                        start=(dt == 0), stop=(dt == D_T - 1),
                    )
                hx = htp.tile([128, 512], bf16, tag="ht")
                nc.scalar.activation(
                    hx[:], u1[:], SILU, bias=sb1[:, ht : ht + 1]
                )
                hsh[ht] = hx
            for tsub in range(TS // 128):
                zt = ypool.tile([128, D], bf16, tag="y")
                for dch in range(D // 512):
                    acc = ps2.tile([128, 512], f32, tag="acc")
                    for ht in range(SH_T):
                        nc.tensor.matmul(
                            acc[:],
                            lhsT=hsh[ht][:, tsub * 128 : (tsub + 1) * 128],
                            rhs=sw2s[ht][:, dch * 512 : (dch + 1) * 512],
                            start=(ht == 0), stop=(ht == SH_T - 1),
                        )
                    nc.vector.tensor_copy(
                        zt[:, dch * 512 : (dch + 1) * 512], acc[:]
                    )
                row = E_LOC * C + tsub * 128
                # scalar HW queue: idle by now, so the tail is DMA-fast
                nc.scalar.dma_start(out_d[row : row + 128, :], zt[:])
    _shrink_redundant_ldw(nc, mybir)
    nc.compile()
    return nc


def _tf(a):
    return np.ascontiguousarray(np.asarray(a, dtype=np.float32))


def _host_gate(emb2d, gate_w):
    """Replicates softmax + top-2 combine coefficients of the reference."""
    logits = (emb2d @ gate_w.T).astype(np.float32)
    m = logits.max(axis=-1, keepdims=True)
    ex = np.exp(logits - m)
    scores = ex / ex.sum(axis=-1, keepdims=True)  # fp32 softmax
    idx = np.argsort(-scores, axis=-1, kind="stable")[:, :2]  # jax tie order
    c = np.zeros((T, E), dtype=np.float32)
    np.put_along_axis(c, idx, np.take_along_axis(scores, idx, axis=-1), axis=-1)
    return c


def _w13_layout(w):  # [D, H_sl] -> [ht, p, dt, h] contiguous blocks
    hsl = w.shape[1]
    return np.ascontiguousarray(
        w.reshape(8, 128, hsl // 128, 128).transpose(2, 1, 0, 3)
    )


def _act_flat(blockT, widths):
    # [D, C] column block -> one flat [128, D_T, w] (partition-major)
    # buffer per si block, so each block is a single contiguous DMA
    parts, base = [], 0
    for w in widths:
        g = np.ascontiguousarray(blockT[:, base : base + w])  # [D, w]
        parts.append(
            np.ascontiguousarray(
                g.reshape(D_T, 128, w).transpose(1, 0, 2)
            ).reshape(-1)
        )
        base += w
    return np.concatenate(parts)


def kernel(embeddings, x, gate_w, W1, B1, W2, B2, W3, B3, sW1, sB1, sW2, sB2):
    global LAST_IN_MAPS
    from concourse.bass_utils import run_bass_kernel_spmd

    embeddings = _tf(embeddings)
    x = _tf(x)
    gate_w, W1, B1, W2, B2, W3, B3 = map(_tf, (gate_w, W1, B1, W2, B2, W3, B3))
    sW1, sB1, sW2, sB2 = map(_tf, (sW1, sB1, sW2, sB2))

    emb2d = embeddings.reshape(T, D)
    embT = np.ascontiguousarray(emb2d.T).astype(BF16)
    xT = np.ascontiguousarray(x.T).astype(BF16)
    c = _host_gate(emb2d, gate_w)

    routed = c > 0.0  # [T, E] exact sparsity mask
    loads = routed.sum(axis=0)
    C = int(max(256, -(-int(loads.max()) // 64) * 64))  # round up to 64
    tsubs = _tsubs_for(C)
    NT = len(tsubs)

    # per-expert gathered token indices, padded with a non-routed token so
    # host scatter-add (unique real indices) stays exact
    idx_all, pad_used = [], []
    for e in range(E):
        idx = np.nonzero(routed[:, e])[0]
        free = np.nonzero(~routed[:, e])[0]
        pad = int(free[0]) if len(free) else 0
        idx_p = np.full(C, pad, dtype=np.int64)
        idx_p[: len(idx)] = idx
        idx_all.append(idx_p)
        pad_used.append(len(idx))

    W1b, W3b = W1.astype(BF16), W3.astype(BF16)
    sw1l = _w13_layout(sW1.astype(BF16))
    sw2b = sW2.astype(BF16)
    sb1l = np.ascontiguousarray(sB1.reshape(SH_T, 128).T)

    in_maps = []
    for core in range(N_CORES):
        e0 = 2 * core
        w1l = np.stack([_w13_layout(W1b[e0 + i]) for i in range(E_LOC)])
        w3l = np.stack([_w13_layout(W3b[e0 + i]) for i in range(E_LOC)])
        w2l = np.ascontiguousarray(W2[e0 : e0 + E_LOC].astype(BF16))
        srcT = xT if core == 0 else embT  # experts 0,1 consume x
        bts = []
        cscc = np.zeros((128, E_LOC * NT), dtype=np.float32)
        for i in range(E_LOC):
            idx = idx_all[e0 + i]
            bts.append(_act_flat(srcT[:, idx], _widths_for(C, head=(i == 0))))
            cv = c[idx, e0 + i].astype(np.float32)
            cv[pad_used[e0 + i] :] = 0.0
            pos = 0
            for j, tw in enumerate(tsubs):
                cscc[:tw, i * NT + j] = cv[pos : pos + tw]
                pos += tw
        b1c = np.ascontiguousarray(
            B1[e0 : e0 + E_LOC].reshape(E_LOC, HT, 128).transpose(2, 0, 1).reshape(128, -1)
        )
        atc = _act_flat(embT[:, core * TS : (core + 1) * TS], [512])
        in_maps.append(
            {
                "bt0": bts[0], "bt1": bts[1], "at": atc,
                "w1": w1l, "w3": w3l, "w2": w2l,
                "sw1": sw1l, "sw2": sw2b, "csc": cscc,
                "b1": b1c, "sb1": sb1l,
            }
        )

    LAST_IN_MAPS = in_maps
    if C not in _CACHED:
        _CACHED[C] = _build(C)
    nc = _CACHED[C]

    res = run_bass_kernel_spmd(nc, in_maps, core_ids=list(range(N_CORES)))

    y = np.zeros((T, D), dtype=np.float32)
    for core in range(N_CORES):
        o = np.asarray(res.results[core]["out"], dtype=np.float32)
        y[core * TS : (core + 1) * TS] += o[E_LOC * C :]  # shared slice
        for i in range(E_LOC):
            # pad rows are exactly zero (c=0) and target a non-routed token
            y[idx_all[2 * core + i]] += o[i * C : (i + 1) * C]
    # host-side exact linear bias terms: sum_e c[t,e]*B2[e,:] and sB2
    y += c @ B2
    y += sB2[None, :]
    return y.reshape(B_DIM, S_DIM, D)


# revision 40
# speedup vs baseline: 1.1737x; 1.1737x over previous
"""MixedMoE Trainium2 kernel: sparse expert routing over 8 NeuronCores.

Reference computation (top-2 of 16 experts, combine weight c[t,e] = softmax
score if e in top-2 else exactly 0):
    emb = embeddings.reshape(T, D)
    experts 0..1 consume x, experts 2..15 consume emb (SwiGLU, inter dim H)
    y[t] = sum_e c[t,e] * expert_e(...)[t]          (c exactly 0 off top-2)
    z = silu(emb @ sW1 + sB1) @ sW2 + sB2           (shared experts, all tokens)
    out = (y + z).reshape(B, S, D)

Because c is exactly zero off the top-2, skipping non-routed (token, expert)
pairs matches the dense reference: we only drop terms that are 0.0 * finite.
The host computes the gate (0.03% of the FLOPs), gathers each expert's routed
tokens, and scatters the expert outputs back.

Sharding (SPMD, one program, per-core data):
  core c holds routed experts {2c, 2c+1}; the host gathers each expert's
  routed tokens (padded to a common capacity C at 64-slot granularity; pad
  slots have c=0) into a [D, C] activation block. The shared experts are
  token-sharded: core c computes the full 2048-wide shared MLP for tokens
  [512c, 512c+512) of emb.

All matmul operands are bf16 (end-to-end error ~3e-3 << the 2e-2 gate):
same 1 cycle/row as f32r, but FWL halves LDWEIGHTS and all DMA halves.
Outputs are bf16 too (host upcasts; the exact-linear bias terms and the
final scatter-add stay fp32 on host). A burst of dummy matmuls at kernel
start warms the PE HAM clock gate while the first DMAs stream; the first
moving block is only 128 columns so real matmuls start early. Outputs go
out over the scalar/gpsimd queues while sync streams weights.
"""

import numpy as np
import ml_dtypes

B_DIM, S_DIM, D = 4, 1024, 1024
T = B_DIM * S_DIM  # 4096 tokens
H = 1024  # routed expert inter dim
E = 16
N_CORES = 8
E_LOC = 2  # routed experts per core
SH = 2048  # shared experts inter dim
SH_T = SH // 128  # 16 shared h-tiles
TS = T // N_CORES  # 512 shared tokens per core
HT = H // 128  # 8 h-tiles per routed expert
D_T = D // 128  # 8 k-tiles in D

BF16 = ml_dtypes.bfloat16

_CACHED = {}  # C -> compiled nc
LAST_IN_MAPS = None  # kept for external timing/debug harnesses


def _tsubs_for(C):
    """Stage-2 token subtiles: 128-wide, plus a trailing 64-wide if C%128."""
    out = [128] * (C // 128)
    if C % 128:
        out.append(64)
    return out


def _widths_for(chunk, head):
    """Stage-1 moving-dim pieces, each <=512 (PSUM bank limit for fp32
    accumulation). All pieces except the last are multiples of 128 so
    stage-2 token subtiles never straddle a piece. The first expert's
    pieces start small ([128, 128, ...]) so the first matmuls only wait
    on ~200KB of DMA and later groups unblock incrementally while the
    activation stream is still ramping."""
    if head and chunk > 128:
        out = [128]
        chunk -= 128
    else:
        out = []
    while chunk > 512:
        out.append(512)
        chunk -= 512
    if chunk:
        out.append(chunk)
    return out


def _shrink_redundant_ldw(nc, mybir):
    """Shrink back-to-back PE weight loads of the same stationary tile.

    bass emits one InstLdweights per matmul. When consecutive matmuls share
    the stationary operand (only matmuls in between), the repeat load is
    redundant -- the PE array already holds those exact weights. The repeat
    instruction must stay (it carries the DMA-wait semaphores that
    move_matmul_waits_to_ldweights later attaches), so instead of deleting
    it we shrink its access pattern to a single column: it reloads PE
    column 0 with identical values (a no-op by value) in ~1 cycle instead
    of 64.
    """
    PE = mybir.EngineType.PE
    n = 0
    for blk in nc.m.functions[0].blocks:
        last_key = None
        for inst in blk.instructions:
            if isinstance(inst, mybir.InstLdweights) and inst.engine == PE:
                ap = inst.ins[0]
                key = str(ap)
                if key == last_key:
                    shr = mybir.PhysicalAccessPattern(
                        kind="physical_ap",
                        ap=[list(ap.ap[0]), [1, 1]],
                        offset=ap.offset, dtype=ap.dtype,
                        memref=ap.memref, memsetref=ap.memsetref,
                    )
                    inst.ins = [shr]
                    n += 1
                else:
                    last_key = key
            elif isinstance(inst, mybir.InstMatmult) and inst.engine == PE:
                pass  # matmuls between identical loads keep the weights hot
            elif inst.engine == PE or isinstance(
                inst, (mybir.InstUnconditionalBranch, mybir.InstCall)
            ):
                last_key = None
    return n


def _build(C):
    import concourse.tile as tile
    from concourse import bacc, mybir

    f32 = mybir.dt.float32
    bf16 = mybir.dt.bfloat16
    SILU = mybir.ActivationFunctionType.Silu
    NT = len(_tsubs_for(C))  # t-subtiles per routed expert

    nc = bacc.Bacc(trn_type="TRN2")

    # ---- DRAM I/O ----
    # activations are stored tile-contiguous (one flat [128*w] block per
    # (si, dt) SBUF tile, in consumption order) so every activation DMA is
    # a single fat contiguous transfer instead of 128 sub-2KB strided lines
    bt0_d = nc.dram_tensor("bt0", [D * C], bf16, kind="ExternalInput")
    bt1_d = nc.dram_tensor("bt1", [D * C], bf16, kind="ExternalInput")
    at_d = nc.dram_tensor("at", [D * TS], bf16, kind="ExternalInput")
    # W1/W3 pre-laid-out per (expert, h_tile): [e, ht, p, dt, h] so each
    # [128, 8, 128] SBUF tile is one fully-contiguous DRAM block
    w1_d = nc.dram_tensor("w1", [E_LOC, HT, 128, 8, 128], bf16, kind="ExternalInput")
    w3_d = nc.dram_tensor("w3", [E_LOC, HT, 128, 8, 128], bf16, kind="ExternalInput")
    w2_d = nc.dram_tensor("w2", [E_LOC, H, D], bf16, kind="ExternalInput")
    sw1_d = nc.dram_tensor("sw1", [SH_T, 128, 8, 128], bf16, kind="ExternalInput")
    sw2_d = nc.dram_tensor("sw2", [SH, D], bf16, kind="ExternalInput")
    # combine scalars csc[p, e*NT + j] = c[token in slot j*128+p, expert e]
    csc_d = nc.dram_tensor("csc", [128, E_LOC * NT], f32, kind="ExternalInput")
    b1_d = nc.dram_tensor("b1", [128, E_LOC * HT], f32, kind="ExternalInput")
    sb1_d = nc.dram_tensor("sb1", [128, SH_T], f32, kind="ExternalInput")
    out_d = nc.dram_tensor("out", [E_LOC * C + TS, D], bf16, kind="ExternalOutput")

    with tile.TileContext(nc) as tc:
        with (
            tc.tile_pool(name="small", bufs=1) as small,
            tc.tile_pool(name="btp", bufs=5) as btp,
            tc.tile_pool(name="w13p", bufs=6) as w13p,
            tc.tile_pool(name="w2p", bufs=17) as w2p,
            tc.tile_pool(name="htp", bufs=18) as htp,
            tc.tile_pool(name="silup", bufs=3) as silup,
            tc.tile_pool(name="yp", bufs=5) as ypool,
            tc.tile_pool(name="ps1", bufs=4, space="PSUM") as ps1,
            tc.tile_pool(name="ps2", bufs=4, space="PSUM") as ps2,
        ):
            csc = small.tile([128, E_LOC * NT], f32)
            b1 = small.tile([128, E_LOC * HT], f32)
            sb1 = small.tile([128, SH_T], f32)
            # HAM warm-up: zeroed operands, dummy matmuls into a scratch
            # PSUM tile; runs while the first DMAs stream so the PE clock
            # gate is already at 2.4 GHz when the real matmuls arrive.
            wu_w = small.tile([128, 128], bf16)
            wu_m = small.tile([128, 512], bf16)
            nc.vector.memset(wu_w[:], 0)
            nc.vector.memset(wu_m[:], 0)
            # 16 x 512: ~3.41us at the cold 1.2 GHz clock un-throttles the
            # HAM clock gate (one full window), and the remaining warm
            # matmuls bridge the PE over the DMA-bound head so the real
            # stream starts warm and nearly gap-free.
            for _ in range(16):
                wu_p = ps2.tile([128, 512], f32, tag="acc")
                nc.tensor.matmul(wu_p[:], lhsT=wu_w[:], rhs=wu_m[:],
                                 start=True, stop=True)

            def load_acts(dram, widths):
                # one [128, 8, w] tile and ONE fat DMA per si block: a
                # single trigger instruction on the scalar ring (each
                # trigger costs ~0.6us of engine time) and one maximally
                # contiguous HBM read
                tiles = []
                off = 0
                for w in widths:
                    t = btp.tile([128, D_T, 512], bf16, tag="bt")
                    nc.scalar.dma_start(
                        t[:, :, :w], dram[off : off + 128 * D_T * w]
                    )
                    tiles.append(t)
                    off += 128 * D_T * w
                return tiles

            def smalls_once():
                nc.sync.dma_start(sb1[:], sb1_d[:])
                nc.sync.dma_start(csc[:], csc_d[:])
                nc.sync.dma_start(b1[:], b1_d[:])

            first = True
            # ---- routed expert phases ----
            for e in range(E_LOC):
                bt_d = (bt0_d, bt1_d)[e]
                widths = _widths_for(C, head=(e == 0))
                if first:
                    smalls_once()
                    # first h-tile's W1/W3 ahead of the activations so the
                    # first real matmul waits on ~300KB, not ~1.5MB
                    w1s_f = w13p.tile([128, 8, 128], bf16, tag="w13")
                    nc.sync.dma_start(w1s_f[:], w1_d[0, 0])
                    w3s_f = w13p.tile([128, 8, 128], bf16, tag="w13")
                    nc.sync.dma_start(w3s_f[:], w3_d[0, 0])
                bts = load_acts(bt_d, widths)
                hts = [[None] * len(widths) for _ in range(HT)]
                w2s = []
                for ht in range(HT):
                    # W2 for this h-tile ahead of its W1/W3 in the scalar
                    # FIFO: issues ~2 h-tile periods early, so stage 2
                    # never waits on the last W2 transfer
                    w2t = w2p.tile([128, D], bf16, tag="w2")
                    nc.scalar.dma_start(
                        w2t[:], w2_d[e, ht * 128 : (ht + 1) * 128, :]
                    )
                    w2s.append(w2t)
                    if first and ht == 0:
                        w1s, w3s = w1s_f, w3s_f
                        first = False
                    else:
                        w1s = w13p.tile([128, 8, 128], bf16, tag="w13")
                        nc.sync.dma_start(w1s[:], w1_d[e, ht])
                        w3s = w13p.tile([128, 8, 128], bf16, tag="w13")
                        nc.sync.dma_start(w3s[:], w3_d[e, ht])
                    for si, w in enumerate(widths):
                        u1 = ps1.tile([128, 512], f32, tag="u")
                        u3 = ps1.tile([128, 512], f32, tag="u")
                        for dt in range(D_T):
                            nc.tensor.matmul(
                                u1[:, :w], lhsT=w1s[:, dt, :],
                                rhs=bts[si][:, dt, :w],
                                start=(dt == 0), stop=(dt == D_T - 1),
                            )
                        for dt in range(D_T):
                            nc.tensor.matmul(
                                u3[:, :w], lhsT=w3s[:, dt, :],
                                rhs=bts[si][:, dt, :w],
                                start=(dt == 0), stop=(dt == D_T - 1),
                            )
                        sil = silup.tile([128, 512], f32, tag="sil")
                        nc.scalar.activation(
                            sil[:, :w], u1[:, :w], SILU,
                            bias=b1[:, e * HT + ht : e * HT + ht + 1],
                        )
                        hx = htp.tile([128, 512], bf16, tag="ht")
                        nc.vector.tensor_mul(hx[:, :w], sil[:, :w], u3[:, :w])
                        hts[ht][si] = hx

                # tsub -> (sub index, col offset inside that sub)
                tmap, pos = [], 0
                for tw in _tsubs_for(C):
                    acc_w, si = 0, 0
                    while acc_w + widths[si] <= pos:
                        acc_w += widths[si]
                        si += 1
                    tmap.append((si, pos - acc_w, tw))
                    pos += tw
                for tsub, (si, off, tw) in enumerate(tmap):
                    g = e * NT + tsub
                    yt = ypool.tile([128, D], bf16, tag="y")
                    for dch in range(D // 512):
                        acc = ps2.tile([128, 512], f32, tag="acc")
                        for ht in range(HT):
                            nc.tensor.matmul(
                                acc[:tw, :],
                                lhsT=hts[ht][si][:, off : off + tw],
                                rhs=w2s[ht][:, dch * 512 : (dch + 1) * 512],
                                start=(ht == 0), stop=(ht == HT - 1),
                            )
                        nc.vector.tensor_scalar_mul(
                            yt[:tw, dch * 512 : (dch + 1) * 512],
                            acc[:tw, :], csc[:tw, g : g + 1],
                        )
                    row = e * C + tsub * 128
                    nc.gpsimd.dma_start(out_d[row : row + tw, :], yt[:tw, :])

            # ---- shared expert phase (512 tokens, full 2048 hidden) ----
            ats = load_acts(at_d, [512])
            sw2s = []
            hsh = [None] * SH_T
            for ht in range(SH_T):
                w2t = w2p.tile([128, D], bf16, tag="w2")
                nc.sync.dma_start(w2t[:], sw2_d[ht * 128 : (ht + 1) * 128, :])
                sw2s.append(w2t)
                w1s = w13p.tile([128, 8, 128], bf16, tag="w13")
                nc.sync.dma_start(w1s[:], sw1_d[ht])
                u1 = ps1.tile([128, 512], f32, tag="u")
                for dt in range(D_T):
                    nc.tensor.matmul(
                        u1[:], lhsT=w1s[:, dt, :], rhs=ats[0][:, dt, :],
                        start=(dt == 0), stop=(dt == D_T - 1),
                    )
                hx = htp.tile([128, 512], bf16, tag="ht")
                nc.scalar.activation(
                    hx[:], u1[:], SILU, bias=sb1[:, ht : ht + 1]
                )
                hsh[ht] = hx
            for tsub in range(TS // 128):
                zt = ypool.tile([128, D], bf16, tag="y")
                row = E_LOC * C + tsub * 128
                for dch in range(D // 512):
                    acc = ps2.tile([128, 512], f32, tag="acc")
                    for ht in range(SH_T):
                        nc.tensor.matmul(
                            acc[:],
                            lhsT=hsh[ht][:, tsub * 128 : (tsub + 1) * 128],
                            rhs=sw2s[ht][:, dch * 512 : (dch + 1) * 512],
                            start=(ht == 0), stop=(ht == SH_T - 1),
                        )
                    nc.vector.tensor_copy(
                        zt[:, dch * 512 : (dch + 1) * 512], acc[:]
                    )
                    # per-half DMA on the idle scalar HW queue: the first
                    # half ships while the second half's matmuls run, so
                    # the end-of-kernel DMA tail halves
                    nc.scalar.dma_start(
                        out_d[row : row + 128, dch * 512 : (dch + 1) * 512],
                        zt[:, dch * 512 : (dch + 1) * 512],
                    )
    _shrink_redundant_ldw(nc, mybir)
    nc.compile()
    return nc


def _tf(a):
    return np.ascontiguousarray(np.asarray(a, dtype=np.float32))


def _host_gate(emb2d, gate_w):
    """Replicates softmax + top-2 combine coefficients of the reference."""
    logits = (emb2d @ gate_w.T).astype(np.float32)
    m = logits.max(axis=-1, keepdims=True)
    ex = np.exp(logits - m)
    scores = ex / ex.sum(axis=-1, keepdims=True)  # fp32 softmax
    idx = np.argsort(-scores, axis=-1, kind="stable")[:, :2]  # jax tie order
    c = np.zeros((T, E), dtype=np.float32)
    np.put_along_axis(c, idx, np.take_along_axis(scores, idx, axis=-1), axis=-1)
    return c


def _w13_layout(w):  # [D, H_sl] -> [ht, p, dt, h] contiguous blocks
    hsl = w.shape[1]
    return np.ascontiguousarray(
        w.reshape(8, 128, hsl // 128, 128).transpose(2, 1, 0, 3)
    )


def _act_flat(blockT, widths):
    # [D, C] column block -> one flat [128, D_T, w] (partition-major)
    # buffer per si block, so each block is a single contiguous DMA
    parts, base = [], 0
    for w in widths:
        g = np.ascontiguousarray(blockT[:, base : base + w])  # [D, w]
        parts.append(
            np.ascontiguousarray(
                g.reshape(D_T, 128, w).transpose(1, 0, 2)
            ).reshape(-1)
        )
        base += w
    return np.concatenate(parts)


def kernel(embeddings, x, gate_w, W1, B1, W2, B2, W3, B3, sW1, sB1, sW2, sB2):
    global LAST_IN_MAPS
    from concourse.bass_utils import run_bass_kernel_spmd

    embeddings = _tf(embeddings)
    x = _tf(x)
    gate_w, W1, B1, W2, B2, W3, B3 = map(_tf, (gate_w, W1, B1, W2, B2, W3, B3))
    sW1, sB1, sW2, sB2 = map(_tf, (sW1, sB1, sW2, sB2))

    emb2d = embeddings.reshape(T, D)
    embT = np.ascontiguousarray(emb2d.T).astype(BF16)
    xT = np.ascontiguousarray(x.T).astype(BF16)
    c = _host_gate(emb2d, gate_w)

    routed = c > 0.0  # [T, E] exact sparsity mask
    loads = routed.sum(axis=0)
    C = int(max(256, -(-int(loads.max()) // 64) * 64))  # round up to 64
    tsubs = _tsubs_for(C)
    NT = len(tsubs)

    # per-expert gathered token indices, padded with a non-routed token so
    # host scatter-add (unique real indices) stays exact
    idx_all, pad_used = [], []
    for e in range(E):
        idx = np.nonzero(routed[:, e])[0]
        free = np.nonzero(~routed[:, e])[0]
        pad = int(free[0]) if len(free) else 0
        idx_p = np.full(C, pad, dtype=np.int64)
        idx_p[: len(idx)] = idx
        idx_all.append(idx_p)
        pad_used.append(len(idx))

    W1b, W3b = W1.astype(BF16), W3.astype(BF16)
    sw1l = _w13_layout(sW1.astype(BF16))
    sw2b = sW2.astype(BF16)
    sb1l = np.ascontiguousarray(sB1.reshape(SH_T, 128).T)

    in_maps = []
    for core in range(N_CORES):
        e0 = 2 * core
        w1l = np.stack([_w13_layout(W1b[e0 + i]) for i in range(E_LOC)])
        w3l = np.stack([_w13_layout(W3b[e0 + i]) for i in range(E_LOC)])
        w2l = np.ascontiguousarray(W2[e0 : e0 + E_LOC].astype(BF16))
        srcT = xT if core == 0 else embT  # experts 0,1 consume x
        bts = []
        cscc = np.zeros((128, E_LOC * NT), dtype=np.float32)
        for i in range(E_LOC):
            idx = idx_all[e0 + i]
            bts.append(_act_flat(srcT[:, idx], _widths_for(C, head=(i == 0))))
            cv = c[idx, e0 + i].astype(np.float32)
            cv[pad_used[e0 + i] :] = 0.0
            pos = 0
            for j, tw in enumerate(tsubs):
                cscc[:tw, i * NT + j] = cv[pos : pos + tw]
                pos += tw
        b1c = np.ascontiguousarray(
            B1[e0 : e0 + E_LOC].reshape(E_LOC, HT, 128).transpose(2, 0, 1).reshape(128, -1)
        )
        atc = _act_flat(embT[:, core * TS : (core + 1) * TS], [512])
        in_maps.append(
            {
                "bt0": bts[0], "bt1": bts[1], "at": atc,
                "w1": w1l, "w3": w3l, "w2": w2l,
                "sw1": sw1l, "sw2": sw2b, "csc": cscc,
                "b1": b1c, "sb1": sb1l,
            }
        )

    LAST_IN_MAPS = in_maps
    if C not in _CACHED:
        _CACHED[C] = _build(C)
    nc = _CACHED[C]

    res = run_bass_kernel_spmd(nc, in_maps, core_ids=list(range(N_CORES)))

    y = np.zeros((T, D), dtype=np.float32)
    for core in range(N_CORES):
        o = np.asarray(res.results[core]["out"], dtype=np.float32)
        y[core * TS : (core + 1) * TS] += o[E_LOC * C :]  # shared slice
        for i in range(E_LOC):
            # pad rows are exactly zero (c=0) and target a non-routed token
            y[idx_all[2 * core + i]] += o[i * C : (i + 1) * C]
    # host-side exact linear bias terms: sum_e c[t,e]*B2[e,:] and sB2
    y += c @ B2
    y += sB2[None, :]
    return y.reshape(B_DIM, S_DIM, D)


# revision 41
# speedup vs baseline: 1.1896x; 1.0135x over previous
"""MixedMoE Trainium2 kernel: sparse expert routing over 8 NeuronCores.

Reference computation (top-2 of 16 experts, combine weight c[t,e] = softmax
score if e in top-2 else exactly 0):
    emb = embeddings.reshape(T, D)
    experts 0..1 consume x, experts 2..15 consume emb (SwiGLU, inter dim H)
    y[t] = sum_e c[t,e] * expert_e(...)[t]          (c exactly 0 off top-2)
    z = silu(emb @ sW1 + sB1) @ sW2 + sB2           (shared experts, all tokens)
    out = (y + z).reshape(B, S, D)

Because c is exactly zero off the top-2, skipping non-routed (token, expert)
pairs matches the dense reference: we only drop terms that are 0.0 * finite.
The host computes the gate (0.03% of the FLOPs), gathers each expert's routed
tokens, and scatters the expert outputs back.

Sharding (SPMD, one program, per-core data):
  core c holds routed experts {2c, 2c+1}; the host gathers each expert's
  routed tokens (padded to a common capacity C at 64-slot granularity; pad
  slots have c=0) into a [D, C] activation block. The shared experts are
  token-sharded: core c computes the full 2048-wide shared MLP for tokens
  [512c, 512c+512) of emb.

All matmul operands are bf16 (end-to-end error ~3e-3 << the 2e-2 gate):
same 1 cycle/row as f32r, but FWL halves LDWEIGHTS and all DMA halves.
Outputs are bf16 too (host upcasts; the exact-linear bias terms and the
final scatter-add stay fp32 on host). A burst of dummy matmuls at kernel
start warms the PE HAM clock gate while the first DMAs stream; the first
moving block is only 128 columns so real matmuls start early. Outputs go
out over the scalar/gpsimd queues while sync streams weights.
"""

import numpy as np
import ml_dtypes

B_DIM, S_DIM, D = 4, 1024, 1024
T = B_DIM * S_DIM  # 4096 tokens
H = 1024  # routed expert inter dim
E = 16
N_CORES = 8
E_LOC = 2  # routed experts per core
SH = 2048  # shared experts inter dim
SH_T = SH // 128  # 16 shared h-tiles
TS = T // N_CORES  # 512 shared tokens per core
HT = H // 128  # 8 h-tiles per routed expert
D_T = D // 128  # 8 k-tiles in D

BF16 = ml_dtypes.bfloat16

_CACHED = {}  # C -> compiled nc
LAST_IN_MAPS = None  # kept for external timing/debug harnesses


def _tsubs_for(C):
    """Stage-2 token subtiles: 128-wide, plus a trailing 64-wide if C%128."""
    out = [128] * (C // 128)
    if C % 128:
        out.append(64)
    return out


def _widths_for(chunk, head):
    """Stage-1 moving-dim pieces, each <=512 (PSUM bank limit for fp32
    accumulation). All pieces except the last are multiples of 128 so
    stage-2 token subtiles never straddle a piece. The first expert's
    pieces start small ([128, 128, ...]) so the first matmuls only wait
    on ~200KB of DMA and later groups unblock incrementally while the
    activation stream is still ramping."""
    out = []
    del head  # warm-up matmuls cover the kernel head now; uniform pieces
    while chunk > 512:
        out.append(512)
        chunk -= 512
    if chunk:
        out.append(chunk)
    return out


def _shrink_redundant_ldw(nc, mybir):
    """Shrink back-to-back PE weight loads of the same stationary tile.

    bass emits one InstLdweights per matmul. When consecutive matmuls share
    the stationary operand (only matmuls in between), the repeat load is
    redundant -- the PE array already holds those exact weights. The repeat
    instruction must stay (it carries the DMA-wait semaphores that
    move_matmul_waits_to_ldweights later attaches), so instead of deleting
    it we shrink its access pattern to a single column: it reloads PE
    column 0 with identical values (a no-op by value) in ~1 cycle instead
    of 64.
    """
    PE = mybir.EngineType.PE
    n = 0
    for blk in nc.m.functions[0].blocks:
        last_key = None
        for inst in blk.instructions:
            if isinstance(inst, mybir.InstLdweights) and inst.engine == PE:
                ap = inst.ins[0]
                key = str(ap)
                if key == last_key:
                    shr = mybir.PhysicalAccessPattern(
                        kind="physical_ap",
                        ap=[list(ap.ap[0]), [1, 1]],
                        offset=ap.offset, dtype=ap.dtype,
                        memref=ap.memref, memsetref=ap.memsetref,
                    )
                    inst.ins = [shr]
                    n += 1
                else:
                    last_key = key
            elif isinstance(inst, mybir.InstMatmult) and inst.engine == PE:
                pass  # matmuls between identical loads keep the weights hot
            elif inst.engine == PE or isinstance(
                inst, (mybir.InstUnconditionalBranch, mybir.InstCall)
            ):
                last_key = None
    return n


def _build(C):
    import concourse.tile as tile
    from concourse import bacc, mybir

    f32 = mybir.dt.float32
    bf16 = mybir.dt.bfloat16
    SILU = mybir.ActivationFunctionType.Silu
    NT = len(_tsubs_for(C))  # t-subtiles per routed expert

    nc = bacc.Bacc(trn_type="TRN2")

    # ---- DRAM I/O ----
    # activations are stored tile-contiguous (one flat [128*w] block per
    # (si, dt) SBUF tile, in consumption order) so every activation DMA is
    # a single fat contiguous transfer instead of 128 sub-2KB strided lines
    bt0_d = nc.dram_tensor("bt0", [D * C], bf16, kind="ExternalInput")
    bt1_d = nc.dram_tensor("bt1", [D * C], bf16, kind="ExternalInput")
    at_d = nc.dram_tensor("at", [D * TS], bf16, kind="ExternalInput")
    # W1/W3 pre-laid-out per (expert, h_tile): [e, ht, p, dt, h] so each
    # [128, 8, 128] SBUF tile is one fully-contiguous DRAM block
    w1_d = nc.dram_tensor("w1", [E_LOC, HT, 128, 8, 128], bf16, kind="ExternalInput")
    w3_d = nc.dram_tensor("w3", [E_LOC, HT, 128, 8, 128], bf16, kind="ExternalInput")
    w2_d = nc.dram_tensor("w2", [E_LOC, H, D], bf16, kind="ExternalInput")
    sw1_d = nc.dram_tensor("sw1", [SH_T, 128, 8, 128], bf16, kind="ExternalInput")
    sw2_d = nc.dram_tensor("sw2", [SH, D], bf16, kind="ExternalInput")
    # combine scalars csc[p, e*NT + j] = c[token in slot j*128+p, expert e]
    csc_d = nc.dram_tensor("csc", [128, E_LOC * NT], f32, kind="ExternalInput")
    b1_d = nc.dram_tensor("b1", [128, E_LOC * HT], f32, kind="ExternalInput")
    sb1_d = nc.dram_tensor("sb1", [128, SH_T], f32, kind="ExternalInput")
    out_d = nc.dram_tensor("out", [E_LOC * C + TS, D], bf16, kind="ExternalOutput")

    with tile.TileContext(nc) as tc:
        with (
            tc.tile_pool(name="small", bufs=1) as small,
            tc.tile_pool(name="btp", bufs=5) as btp,
            tc.tile_pool(name="w13p", bufs=6) as w13p,
            tc.tile_pool(name="w2p", bufs=17) as w2p,
            tc.tile_pool(name="htp", bufs=18) as htp,
            tc.tile_pool(name="silup", bufs=3) as silup,
            tc.tile_pool(name="yp", bufs=5) as ypool,
            tc.tile_pool(name="ps1", bufs=4, space="PSUM") as ps1,
            tc.tile_pool(name="ps2", bufs=4, space="PSUM") as ps2,
        ):
            csc = small.tile([128, E_LOC * NT], f32)
            b1 = small.tile([128, E_LOC * HT], f32)
            sb1 = small.tile([128, SH_T], f32)
            # HAM warm-up: zeroed operands, dummy matmuls into a scratch
            # PSUM tile; runs while the first DMAs stream so the PE clock
            # gate is already at 2.4 GHz when the real matmuls arrive.
            wu_w = small.tile([128, 128], bf16)
            wu_m = small.tile([128, 512], bf16)
            nc.vector.memset(wu_w[:], 0)
            nc.vector.memset(wu_m[:], 0)
            # 16 x 512: ~3.41us at the cold 1.2 GHz clock un-throttles the
            # HAM clock gate (one full window), and the remaining warm
            # matmuls bridge the PE over the DMA-bound head so the real
            # stream starts warm and nearly gap-free.
            for _ in range(16):
                wu_p = ps2.tile([128, 512], f32, tag="acc")
                nc.tensor.matmul(wu_p[:], lhsT=wu_w[:], rhs=wu_m[:],
                                 start=True, stop=True)

            def load_acts(dram, widths):
                # one [128, 8, w] tile and ONE fat DMA per si block: a
                # single trigger instruction on the scalar ring (each
                # trigger costs ~0.6us of engine time) and one maximally
                # contiguous HBM read
                tiles = []
                off = 0
                for w in widths:
                    t = btp.tile([128, D_T, 512], bf16, tag="bt")
                    nc.scalar.dma_start(
                        t[:, :, :w], dram[off : off + 128 * D_T * w]
                    )
                    tiles.append(t)
                    off += 128 * D_T * w
                return tiles

            def smalls_once():
                nc.sync.dma_start(sb1[:], sb1_d[:])
                nc.sync.dma_start(csc[:], csc_d[:])
                nc.sync.dma_start(b1[:], b1_d[:])

            first = True
            # ---- routed expert phases ----
            for e in range(E_LOC):
                bt_d = (bt0_d, bt1_d)[e]
                widths = _widths_for(C, head=(e == 0))
                if first:
                    smalls_once()
                    # first h-tile's W1/W3 ahead of the activations so the
                    # first real matmul waits on ~300KB, not ~1.5MB
                    w1s_f = w13p.tile([128, 8, 128], bf16, tag="w13")
                    nc.sync.dma_start(w1s_f[:], w1_d[0, 0])
                    w3s_f = w13p.tile([128, 8, 128], bf16, tag="w13")
                    nc.sync.dma_start(w3s_f[:], w3_d[0, 0])
                bts = load_acts(bt_d, widths)
                hts = [[None] * len(widths) for _ in range(HT)]
                w2s = []
                for ht in range(HT):
                    # W2 for this h-tile ahead of its W1/W3 in the scalar
                    # FIFO: issues ~2 h-tile periods early, so stage 2
                    # never waits on the last W2 transfer
                    w2t = w2p.tile([128, D], bf16, tag="w2")
                    nc.scalar.dma_start(
                        w2t[:], w2_d[e, ht * 128 : (ht + 1) * 128, :]
                    )
                    w2s.append(w2t)
                    if first and ht == 0:
                        w1s, w3s = w1s_f, w3s_f
                        first = False
                    else:
                        w1s = w13p.tile([128, 8, 128], bf16, tag="w13")
                        nc.sync.dma_start(w1s[:], w1_d[e, ht])
                        w3s = w13p.tile([128, 8, 128], bf16, tag="w13")
                        nc.sync.dma_start(w3s[:], w3_d[e, ht])
                    for si, w in enumerate(widths):
                        u1 = ps1.tile([128, 512], f32, tag="u")
                        u3 = ps1.tile([128, 512], f32, tag="u")
                        for dt in range(D_T):
                            nc.tensor.matmul(
                                u1[:, :w], lhsT=w1s[:, dt, :],
                                rhs=bts[si][:, dt, :w],
                                start=(dt == 0), stop=(dt == D_T - 1),
                            )
                        for dt in range(D_T):
                            nc.tensor.matmul(
                                u3[:, :w], lhsT=w3s[:, dt, :],
                                rhs=bts[si][:, dt, :w],
                                start=(dt == 0), stop=(dt == D_T - 1),
                            )
                        sil = silup.tile([128, 512], f32, tag="sil")
                        nc.scalar.activation(
                            sil[:, :w], u1[:, :w], SILU,
                            bias=b1[:, e * HT + ht : e * HT + ht + 1],
                        )
                        hx = htp.tile([128, 512], bf16, tag="ht")
                        nc.vector.tensor_mul(hx[:, :w], sil[:, :w], u3[:, :w])
                        hts[ht][si] = hx

                # tsub -> (sub index, col offset inside that sub)
                tmap, pos = [], 0
                for tw in _tsubs_for(C):
                    acc_w, si = 0, 0
                    while acc_w + widths[si] <= pos:
                        acc_w += widths[si]
                        si += 1
                    tmap.append((si, pos - acc_w, tw))
                    pos += tw
                for tsub, (si, off, tw) in enumerate(tmap):
                    g = e * NT + tsub
                    yt = ypool.tile([128, D], bf16, tag="y")
                    for dch in range(D // 512):
                        acc = ps2.tile([128, 512], f32, tag="acc")
                        for ht in range(HT):
                            nc.tensor.matmul(
                                acc[:tw, :],
                                lhsT=hts[ht][si][:, off : off + tw],
                                rhs=w2s[ht][:, dch * 512 : (dch + 1) * 512],
                                start=(ht == 0), stop=(ht == HT - 1),
                            )
                        nc.vector.tensor_scalar_mul(
                            yt[:tw, dch * 512 : (dch + 1) * 512],
                            acc[:tw, :], csc[:tw, g : g + 1],
                        )
                    row = e * C + tsub * 128
                    nc.gpsimd.dma_start(out_d[row : row + tw, :], yt[:tw, :])

            # ---- shared expert phase (512 tokens, full 2048 hidden) ----
            ats = load_acts(at_d, [512])
            sw2s = []
            hsh = [None] * SH_T
            for ht in range(SH_T):
                w2t = w2p.tile([128, D], bf16, tag="w2")
                nc.sync.dma_start(w2t[:], sw2_d[ht * 128 : (ht + 1) * 128, :])
                sw2s.append(w2t)
                w1s = w13p.tile([128, 8, 128], bf16, tag="w13")
                nc.sync.dma_start(w1s[:], sw1_d[ht])
                u1 = ps1.tile([128, 512], f32, tag="u")
                for dt in range(D_T):
                    nc.tensor.matmul(
                        u1[:], lhsT=w1s[:, dt, :], rhs=ats[0][:, dt, :],
                        start=(dt == 0), stop=(dt == D_T - 1),
                    )
                hx = htp.tile([128, 512], bf16, tag="ht")
                nc.scalar.activation(
                    hx[:], u1[:], SILU, bias=sb1[:, ht : ht + 1]
                )
                hsh[ht] = hx
            for tsub in range(TS // 128):
                zt = ypool.tile([128, D], bf16, tag="y")
                row = E_LOC * C + tsub * 128
                for dch in range(D // 512):
                    acc = ps2.tile([128, 512], f32, tag="acc")
                    for ht in range(SH_T):
                        nc.tensor.matmul(
                            acc[:],
                            lhsT=hsh[ht][:, tsub * 128 : (tsub + 1) * 128],
                            rhs=sw2s[ht][:, dch * 512 : (dch + 1) * 512],
                            start=(ht == 0), stop=(ht == SH_T - 1),
                        )
                    nc.vector.tensor_copy(
                        zt[:, dch * 512 : (dch + 1) * 512], acc[:]
                    )
                    # per-half DMA on the idle scalar HW queue: the first
                    # half ships while the second half's matmuls run, so
                    # the end-of-kernel DMA tail halves
                    nc.scalar.dma_start(
                        out_d[row : row + 128, dch * 512 : (dch + 1) * 512],
                        zt[:, dch * 512 : (dch + 1) * 512],
                    )
    _shrink_redundant_ldw(nc, mybir)
    nc.compile()
    return nc


def _tf(a):
    return np.ascontiguousarray(np.asarray(a, dtype=np.float32))


def _host_gate(emb2d, gate_w):
    """Replicates softmax + top-2 combine coefficients of the reference."""
    logits = (emb2d @ gate_w.T).astype(np.float32)
    m = logits.max(axis=-1, keepdims=True)
    ex = np.exp(logits - m)
    scores = ex / ex.sum(axis=-1, keepdims=True)  # fp32 softmax
    idx = np.argsort(-scores, axis=-1, kind="stable")[:, :2]  # jax tie order
    c = np.zeros((T, E), dtype=np.float32)
    np.put_along_axis(c, idx, np.take_along_axis(scores, idx, axis=-1), axis=-1)
    return c


def _w13_layout(w):  # [D, H_sl] -> [ht, p, dt, h] contiguous blocks
    hsl = w.shape[1]
    return np.ascontiguousarray(
        w.reshape(8, 128, hsl // 128, 128).transpose(2, 1, 0, 3)
    )


def _act_flat(blockT, widths):
    # [D, C] column block -> one flat [128, D_T, w] (partition-major)
    # buffer per si block, so each block is a single contiguous DMA
    parts, base = [], 0
    for w in widths:
        g = np.ascontiguousarray(blockT[:, base : base + w])  # [D, w]
        parts.append(
            np.ascontiguousarray(
                g.reshape(D_T, 128, w).transpose(1, 0, 2)
            ).reshape(-1)
        )
        base += w
    return np.concatenate(parts)


def kernel(embeddings, x, gate_w, W1, B1, W2, B2, W3, B3, sW1, sB1, sW2, sB2):
    global LAST_IN_MAPS
    from concourse.bass_utils import run_bass_kernel_spmd

    embeddings = _tf(embeddings)
    x = _tf(x)
    gate_w, W1, B1, W2, B2, W3, B3 = map(_tf, (gate_w, W1, B1, W2, B2, W3, B3))
    sW1, sB1, sW2, sB2 = map(_tf, (sW1, sB1, sW2, sB2))

    emb2d = embeddings.reshape(T, D)
    embT = np.ascontiguousarray(emb2d.T).astype(BF16)
    xT = np.ascontiguousarray(x.T).astype(BF16)
    c = _host_gate(emb2d, gate_w)

    routed = c > 0.0  # [T, E] exact sparsity mask
    loads = routed.sum(axis=0)
    C = int(max(256, -(-int(loads.max()) // 64) * 64))  # round up to 64
    tsubs = _tsubs_for(C)
    NT = len(tsubs)

    # per-expert gathered token indices, padded with a non-routed token so
    # host scatter-add (unique real indices) stays exact
    idx_all, pad_used = [], []
    for e in range(E):
        idx = np.nonzero(routed[:, e])[0]
        free = np.nonzero(~routed[:, e])[0]
        pad = int(free[0]) if len(free) else 0
        idx_p = np.full(C, pad, dtype=np.int64)
        idx_p[: len(idx)] = idx
        idx_all.append(idx_p)
        pad_used.append(len(idx))

    W1b, W3b = W1.astype(BF16), W3.astype(BF16)
    sw1l = _w13_layout(sW1.astype(BF16))
    sw2b = sW2.astype(BF16)
    sb1l = np.ascontiguousarray(sB1.reshape(SH_T, 128).T)

    in_maps = []
    for core in range(N_CORES):
        e0 = 2 * core
        w1l = np.stack([_w13_layout(W1b[e0 + i]) for i in range(E_LOC)])
        w3l = np.stack([_w13_layout(W3b[e0 + i]) for i in range(E_LOC)])
        w2l = np.ascontiguousarray(W2[e0 : e0 + E_LOC].astype(BF16))
        srcT = xT if core == 0 else embT  # experts 0,1 consume x
        bts = []
        cscc = np.zeros((128, E_LOC * NT), dtype=np.float32)
        for i in range(E_LOC):
            idx = idx_all[e0 + i]
            bts.append(_act_flat(srcT[:, idx], _widths_for(C, head=(i == 0))))
            cv = c[idx, e0 + i].astype(np.float32)
            cv[pad_used[e0 + i] :] = 0.0
            pos = 0
            for j, tw in enumerate(tsubs):
                cscc[:tw, i * NT + j] = cv[pos : pos + tw]
                pos += tw
        b1c = np.ascontiguousarray(
            B1[e0 : e0 + E_LOC].reshape(E_LOC, HT, 128).transpose(2, 0, 1).reshape(128, -1)
        )
        atc = _act_flat(embT[:, core * TS : (core + 1) * TS], [512])
        in_maps.append(
            {
                "bt0": bts[0], "bt1": bts[1], "at": atc,
                "w1": w1l, "w3": w3l, "w2": w2l,
                "sw1": sw1l, "sw2": sw2b, "csc": cscc,
                "b1": b1c, "sb1": sb1l,
            }
        )

    LAST_IN_MAPS = in_maps
    if C not in _CACHED:
        _CACHED[C] = _build(C)
    nc = _CACHED[C]

    res = run_bass_kernel_spmd(nc, in_maps, core_ids=list(range(N_CORES)))

    y = np.zeros((T, D), dtype=np.float32)
    for core in range(N_CORES):
        o = np.asarray(res.results[core]["out"], dtype=np.float32)
        y[core * TS : (core + 1) * TS] += o[E_LOC * C :]  # shared slice
        for i in range(E_LOC):
            # pad rows are exactly zero (c=0) and target a non-routed token
            y[idx_all[2 * core + i]] += o[i * C : (i + 1) * C]
    # host-side exact linear bias terms: sum_e c[t,e]*B2[e,:] and sB2
    y += c @ B2
    y += sB2[None, :]
    return y.reshape(B_DIM, S_DIM, D)
